# revision 27
# baseline (speedup 1.0000x reference)
"""Causal self-attention (RoPE) Trainium2 kernel.

Problem: B=4, T=2048, D=1024, H=16 heads (hd=64), fp32.
  q,k,v = x@W{q,k,v}.T + b;  rope(q), rope(k);  causal softmax attention;
  y = att_out @ Wo.T + bo.

Sharding (8 cores): data parallel over batch (4), tensor parallel over
heads (2 groups of 8 heads). Core c handles batch c//2, head-group c%2.
Each core computes its 8 heads end-to-end plus the partial out-projection;
the host sums the two head-group partials per batch and adds bo.

On-device layout is transposed ([dim, time]) so that attention matmuls get
the contraction dim (head dim / keys) on partitions:
  - x.T built via PE transposes
  - Q.T/K.T = W @ x.T directly; RoPE applied elementwise, with the
    rotate-half realized as a single +/-1 permutation-matrix matmul on PE
  - V projected per x-strip, interleaved with the transposes (PE filler
    while the next strip's DMA is in flight)
  - S.T = K_h @ Q_h.T per head-PAIR (even head at partitions 0-63, odd at
    64-127 — the two K=64 matmuls run concurrently in the PE array); one
    exp per pair over [128, 1024], diagonal tiles sliced to the valid
    region with a static [128,128] 0/-1e30 triangle added to the S-PSUM
    before exp (keeps at-tiles pure ACT->PE, no event-semaphore storms)
  - O.T = [V_h | 1].T @ A.T accumulated over key tiles; the ones column
    yields the softmax denominator as psum row 64; AV matmuls trail the
    S matmuls by LAG key-tile pairs, and each pair's trailing AVs,
    reciprocals and normalization tails are deferred into the next pair's
    instruction stream so the PE FIFO never drains at pair boundaries
  - normalization via reciprocal + K=1 broadcast matmul
  - out.T = Wo_c.T @ Y.T, emitted per finished query chunk (interleaved
    into the next chunk's attention), written transposed; host transposes

All big matmuls run in float32r (full-rate fp32, ~1e-4 relative rounding).
"""

import sys

sys.path.insert(0, "/opt/trn_rl_repo")

import numpy as np

B, T, D, H = 4, 2048, 1024, 16
HD = 64
ROPE_BASE = 10000.0
N_CORES = 8
HPC = 8  # heads per core
LAG = 5  # AV matmul lag behind S matmul (key-tile pairs)

_cache = {}


def _build_bass():
    import concourse.mybir as mybir
    import concourse.tile as tile
    from concourse import bacc

    f32 = mybir.dt.float32
    f32r = mybir.dt.float32r
    Alu = mybir.AluOpType
    Act = mybir.ActivationFunctionType

    nc = bacc.Bacc()

    # ---- DRAM I/O (per-core shards; same NEFF on all 8 cores) ----
    x_d = nc.dram_tensor("x", [T, D], f32r, kind="ExternalInput")
    wq_d = nc.dram_tensor("wq", [D, 512], f32r, kind="ExternalInput")
    wk_d = nc.dram_tensor("wk", [D, 512], f32r, kind="ExternalInput")
    wv_d = nc.dram_tensor("wv", [D, 512], f32r, kind="ExternalInput")
    wo_d = nc.dram_tensor("wo", [512, D], f32r, kind="ExternalInput")
    bq_d = nc.dram_tensor("bq", [128, 4], f32, kind="ExternalInput")
    bk_d = nc.dram_tensor("bk", [128, 4], f32, kind="ExternalInput")
    bv_d = nc.dram_tensor("bv_bc", [128, 512], f32, kind="ExternalInput")
    cos_d = nc.dram_tensor("cosT", [128, T], f32r, kind="ExternalInput")
    sin_d = nc.dram_tensor("sinT", [128, T], f32r, kind="ExternalInput")
    perm_d = nc.dram_tensor("permT", [128, 128], f32r, kind="ExternalInput")
    tri_d = nc.dram_tensor("triadd", [128, 128], f32, kind="ExternalInput")
    ident_d = nc.dram_tensor("ident", [128, 128], f32r, kind="ExternalInput")
    yt_d = nc.dram_tensor("yT", [D, T], f32, kind="ExternalOutput")

    wq_v = wq_d[:, :].rearrange("(ko p) m -> p ko m", p=128)
    wk_v = wk_d[:, :].rearrange("(ko p) m -> p ko m", p=128)
    wv_v = wv_d[:, :].rearrange("(ko p) m -> p ko m", p=128)
    wo_v = wo_d[:, :].rearrange("(ko p) n -> p ko n", p=128)

    with tile.TileContext(nc) as tc:
        with (
            tc.tile_pool(name="singles", bufs=1) as singles,
            tc.tile_pool(name="big", bufs=1) as big,
        ):
            ident = singles.tile([128, 128], f32r)
            nc.sync.dma_start(ident, ident_d[:, :])
            ones_hi = singles.tile([65, 64], f32)
            nc.vector.memset(ones_hi, 1.0)
            ones128 = singles.tile([128, 16, 8, 1], f32)
            nc.vector.memset(ones128, 1.0)
            # setup constants ride the otherwise-idle gpsimd DMA queue
            # (issued after the wv halves below; consumed only in A2/B)
            bq_sb = singles.tile([128, 4], f32, tag="bq")
            bk_sb = singles.tile([128, 4], f32, tag="bk")
            perm_sb = singles.tile([128, 128], f32r, tag="perm")
            tri_sb = singles.tile([128, 128], f32, tag="tri")

            # persistent activations (f32r so matmuls can consume them)
            qt = big.tile([128, 4, T], f32r, tag="qt")
            kt = big.tile([128, 4, T], f32r, tag="kt")
            v_sb = big.tile([128, 16, HPC, 65], f32r, tag="v")
            # ones columns for the fused softmax-denominator rows: written by
            # DVE (a 4-byte-element gather DMA would clog the DGE ring and
            # block the issuing engine's queue for ~20us)
            nc.vector.tensor_copy(v_sb[:, :, :, 64:65], ones128)

            # ================= Phase A: x.T, Q.T/K.T (roped), V =============
            with (
                tc.tile_pool(name="pa_sb", bufs=1) as pa,
                tc.tile_pool(name="xrow_p", bufs=4) as xrow_p,
                tc.tile_pool(name="wcol_p", bufs=2) as wcol_p,
                tc.tile_pool(name="qa_p", bufs=4) as qa_p,
                tc.tile_pool(name="tmp_p", bufs=2) as tmp_p,
                tc.tile_pool(name="tps", bufs=2, space="PSUM") as tps,
                tc.tile_pool(name="qkps", bufs=3, space="PSUM") as qkps,
                tc.tile_pool(name="auxps", bufs=3, space="PSUM") as auxps,
            ):
                # wv first: the V matmuls of strip 0 sit early in the PE
                # FIFO and must not wait behind the cos/sin table DMAs
                wv_sb = pa.tile([128, 8, 512], f32r, tag="wv")
                nc.scalar.dma_start(wv_sb[:, 0:4, :], wv_v[:, 0:4, :])
                nc.gpsimd.dma_start(wv_sb[:, 4:8, :], wv_v[:, 4:8, :])
                bv_sb = pa.tile([128, 512], f32, tag="bv")
                nc.gpsimd.dma_start(bv_sb, bv_d[:, :])
                nc.gpsimd.dma_start(bq_sb, bq_d[:, :])
                nc.gpsimd.dma_start(bk_sb, bk_d[:, :])
                nc.gpsimd.dma_start(perm_sb, perm_d[:, :])
                nc.gpsimd.dma_start(tri_sb, tri_d[:, :])
                # cos/sin are first consumed in A2 of th=0; issuing them
                # after the first strips keeps the DMA engines on x rows
                cos_sb = pa.tile([128, T], f32r, tag="cos")
                sin_sb = pa.tile([128, T], f32r, tag="sin")
                cs_issued = [False]

                for th in range(2):
                    t0 = th * 1024
                    xt = pa.tile([128, 8, 1024], f32r, tag="xt")
                    # --- A1: transpose x strips into xt; V projection of
                    # each strip follows immediately (PE filler while the
                    # next strip's DMA is in flight) ---
                    def do_v(tt, th=th, xt=xt):
                        gt = th * 8 + tt
                        psV = auxps.tile([128, 512], f32, tag="aux",
                                         name=f"psV_{gt}")
                        for ko in range(8):
                            nc.tensor.matmul(
                                psV, lhsT=xt[:, ko, tt * 128:(tt + 1) * 128],
                                rhs=wv_sb[:, ko, :],
                                start=(ko == 0), stop=(ko == 7))
                        nc.vector.tensor_tensor(
                            v_sb[:, gt, :, 0:64],
                            psV.rearrange("p (h d) -> p h d", h=HPC),
                            bv_sb.rearrange("p (h d) -> p h d", h=HPC),
                            Alu.add)

                    # V lags the transposes (3 strips on the cold start so
                    # early PE work never waits on the wv DMA)
                    vlag = 3 if th == 0 else 1
                    for tt in range(8):
                        r0 = t0 + tt * 128
                        xrow = xrow_p.tile([128, D], f32r, tag="xrow")
                        nc.sync.dma_start(xrow, x_d[r0:r0 + 128, :])
                        if th == 0 and tt == 5 and not cs_issued[0]:
                            cs_issued[0] = True
                            nc.scalar.dma_start(cos_sb, cos_d[:, :])
                            nc.scalar.dma_start(sin_sb, sin_d[:, :])
                        for ko in range(8):
                            tp = tps.tile([128, 128], f32r, tag="tps")
                            nc.tensor.transpose(
                                tp, xrow[:, ko * 128:(ko + 1) * 128], ident)
                            nc.scalar.copy(
                                xt[:, ko, tt * 128:(tt + 1) * 128], tp)
                        if tt >= vlag:
                            do_v(tt - vlag)
                    for tt in range(8 - vlag, 8):
                        do_v(tt)

                    # --- A2: Q.T and K.T with fused RoPE ---
                    for wview, bcol, dest in (
                        (wq_v, bq_sb, qt),
                        (wk_v, bk_sb, kt),
                    ):
                        for qo in range(4):
                            wcol = wcol_p.tile([128, 8, 128], f32r, tag="wcol")
                            nc.sync.dma_start(
                                wcol, wview[:, :, qo * 128:(qo + 1) * 128])
                            # two 512-chunks in flight so the rot matmul's
                            # DVE dependency hides under the other chunk's
                            # accumulation matmuls
                            ps_l, qa_l = [], []
                            for tcc in range(2):
                                c0 = tcc * 512
                                psA = qkps.tile([128, 512], f32, tag="psA")
                                for ko in range(8):
                                    nc.tensor.matmul(
                                        psA, lhsT=wcol[:, ko, :],
                                        rhs=xt[:, ko, c0:c0 + 512],
                                        start=(ko == 0), stop=(ko == 7))
                                qa = qa_p.tile([128, 512], f32r, tag="qa")
                                nc.vector.tensor_scalar_add(
                                    qa, psA, bcol[:, qo:qo + 1])
                                ps_l.append(psA)
                                qa_l.append(qa)
                            rot_l = []
                            for tcc in range(2):
                                rps = auxps.tile([128, 512], f32, tag="aux")
                                nc.tensor.matmul(
                                    rps, lhsT=perm_sb, rhs=qa_l[tcc],
                                    start=True, stop=True)
                                rot_l.append(rps)
                            for tcc in range(2):
                                ta = t0 + tcc * 512
                                # tmp1 is SBUF-only -> pool engine (gpsimd);
                                # tmp2 reads PSUM so it must stay on DVE
                                tmp1 = tmp_p.tile([128, 512], f32, tag="tmp1")
                                nc.gpsimd.tensor_tensor(
                                    tmp1, qa_l[tcc], cos_sb[:, ta:ta + 512],
                                    Alu.mult)
                                tmp2 = tmp_p.tile([128, 512], f32, tag="tmp2")
                                nc.vector.tensor_tensor(
                                    tmp2, rot_l[tcc], sin_sb[:, ta:ta + 512],
                                    Alu.mult)
                                nc.vector.tensor_tensor(
                                    dest[:, qo, ta:ta + 512], tmp1, tmp2,
                                    Alu.add)


            # ================= Phase B: attention ==========================
            with tc.tile_pool(name="pb_keep", bufs=1) as pb_keep:
                yt = pb_keep.tile([128, 4, T], f32r, tag="yt")
                wo_sb = pb_keep.tile([128, 4, D], f32r, tag="wo")
                nc.scalar.dma_start(wo_sb, wo_v)
                with (
                    tc.tile_pool(name="at_p", bufs=7) as at_p,
                    tc.tile_pool(name="rec_p", bufs=4) as rec_p,
                    tc.tile_pool(name="ytmp_p", bufs=2) as ytmp_p,
                    tc.tile_pool(name="orow_p", bufs=4) as orow_p,
                    tc.tile_pool(name="sps", bufs=2, space="PSUM") as sps,
                    tc.tile_pool(name="ops", bufs=4, space="PSUM") as ops,
                ):
                    def emit_outproj(cj, dos=range(8)):
                        p0 = cj * 512
                        for do in dos:
                            ps2 = sps.tile([128, 1024], f32, tag="sps",
                                           name=f"op_{cj}_{do}")
                            ps = ps2[:, 0:512]
                            for ko in range(4):
                                nc.tensor.matmul(
                                    ps, lhsT=wo_sb[:, ko,
                                                   do * 128:(do + 1) * 128],
                                    rhs=yt[:, ko, p0:p0 + 512],
                                    start=(ko == 0), stop=(ko == 3))
                            orow = orow_p.tile([128, 512], f32, tag="orow")
                            nc.vector.tensor_copy(orow, ps)
                            nc.sync.dma_start(
                                yt_d[do * 128:(do + 1) * 128, p0:p0 + 512],
                                orow)

                    pending = []
                    pending_avs = []
                    pending_recips = []

                    def flush_avs():
                        for fn in pending_avs:
                            fn()
                        pending_avs.clear()
                        for fn in pending_recips:
                            fn()
                        pending_recips.clear()

                    def flush_tails():
                        flush_avs()
                        for fn in pending:
                            fn()
                        pending.clear()

                    for ci in range(4):
                        q0 = ci * 512
                        nkt = 4 * ci + 4
                        for ho in range(4):
                            if ho == 1 and ci > 0:
                                flush_tails()
                                emit_outproj(ci - 1, range(0, 4))
                            elif ho == 2 and ci > 0:
                                emit_outproj(ci - 1, range(4, 8))
                            o_pair = [
                                ops.tile([65, 512], f32, tag="ops",
                                         name=f"ops_{ci}_{ho}_{g_}")
                                for g_ in range(2)]
                            at2s = {}

                            def do_av(k_i, o_pair=o_pair, at2s=at2s, nkt=nkt,
                                      ho=ho, ci=ci):
                                at2 = at2s.pop(k_i)
                                sdx = k_i - 4 * ci
                                f0 = max(0, sdx) * 128
                                for g in range(2):
                                    nc.tensor.matmul(
                                        o_pair[g][:, f0:512],
                                        lhsT=v_sb[:, k_i, 2 * ho + g, :],
                                        rhs=at2[:, g * 512 + f0:
                                                (g + 1) * 512],
                                        start=(k_i == 0),
                                        stop=(k_i == nkt - 1))

                            for kt_i in range(nkt):
                                sdx = kt_i - 4 * ci
                                f0 = max(0, sdx) * 128
                                s_ps2 = sps.tile([128, 1024], f32, tag="sps")
                                for g in range(2):
                                    hp = g * 64
                                    nc.tensor.matmul(
                                        s_ps2[:, g * 512 + f0:(g + 1) * 512],
                                        lhsT=kt[hp:hp + 64, ho,
                                                kt_i * 128:(kt_i + 1) * 128],
                                        rhs=qt[hp:hp + 64, ho,
                                               q0 + f0:q0 + 512],
                                        start=True, stop=True)
                                at2 = at_p.tile([128, 1024], f32r, tag="at")
                                av = at2.rearrange("p (g q) -> p g q", g=2)
                                sv = s_ps2.rearrange("p (g q) -> p g q", g=2)
                                if sdx >= 0:
                                    nc.vector.tensor_tensor(
                                        sv[:, :, f0:f0 + 128],
                                        sv[:, :, f0:f0 + 128],
                                        tri_sb[:, None, :].to_broadcast(
                                            (128, 2, 128)),
                                        Alu.add)
                                nc.scalar.activation(
                                    av[:, :, f0:512], sv[:, :, f0:512],
                                    Act.Exp, scale=0.125)
                                at2s[kt_i] = at2
                                if kt_i == 0:
                                    flush_avs()
                                elif kt_i == 1:
                                    flush_tails()
                                if kt_i >= LAG:
                                    do_av(kt_i - LAG)
                            for k_i in range(max(0, nkt - LAG), nkt):
                                pending_avs.append(
                                    lambda k_i=k_i, do_av=do_av: do_av(k_i))

                            recs = [rec_p.tile([65, 512], f32, tag="rec",
                                               name=f"rec_{ci}_{ho}_{g_}")
                                    for g_ in range(2)]
                            for g in range(2):
                                o_ps = o_pair[g]
                                rec = recs[g]

                                def recip(o_ps=o_ps, rec=rec):
                                    # approx reciprocal computes garbage on
                                    # APs not based at partition 0, so run it
                                    # over all 65 rows (same cost: DVE time
                                    # scales with the free dim only). Rows
                                    # 0-63 are junk and never read; row 64 is
                                    # the softmax denominator.
                                    nc.vector.reciprocal_approx_fast(
                                        out=rec, in_=o_ps)

                                pending_recips.append(recip)

                                def rest(g=g, o_ps=o_ps, rec=rec, ho=ho,
                                         q0=q0, ci=ci):
                                    b_ps = ops.tile(
                                        [64, 512], f32, tag="ops",
                                        name=f"bps_{ci}_{ho}_{g}")
                                    nc.tensor.matmul(
                                        b_ps, lhsT=ones_hi[64:65, :],
                                        rhs=rec[64:65, :],
                                        start=True, stop=True)
                                    b_sb2 = rec_p.tile(
                                        [64, 512], f32, tag="bsb")
                                    nc.vector.tensor_copy(b_sb2, b_ps)
                                    if g == 0:
                                        nc.vector.tensor_tensor(
                                            yt[0:64, ho, q0:q0 + 512],
                                            o_ps[0:64, :], b_sb2, Alu.mult)
                                    else:
                                        ytmp = ytmp_p.tile(
                                            [64, 512], f32r, tag="ytmp")
                                        nc.vector.tensor_tensor(
                                            ytmp, o_ps[0:64, :], b_sb2,
                                            Alu.mult)
                                        nc.sync.dma_start(
                                            yt[64:128, ho, q0:q0 + 512], ytmp)

                                pending.append(rest)

                    flush_tails()
                    emit_outproj(3)

    nc.finalize()
    return nc


def _prep_shards(x, Wq, bq, Wk, bk, Wv, bv, Wo, bo):
    f = np.float32
    theta = 1.0 / (ROPE_BASE ** (np.arange(0, HD, 2, dtype=f) / HD))  # [32]
    pos = np.arange(1, T + 1, dtype=f)
    ang = pos[:, None] * theta[None, :]  # [T, 32]
    j = (np.arange(128) % HD) % 32
    cosT = np.ascontiguousarray(np.cos(ang).T[j, :], dtype=f)  # [128, T]
    sinT = np.ascontiguousarray(np.sin(ang).T[j, :], dtype=f)
    # rotate-half permutation (with sign): rot[p] = sgn(p) * q[p ^ 32]
    prm = np.zeros((128, 128), dtype=f)
    pp = np.arange(128)
    prm[pp, pp ^ 32] = np.where((pp % HD) < 32, -1.0, 1.0)
    permT = np.ascontiguousarray(prm.T)

    # additive causal mask for the diagonal 128-block: keep c >= p
    cc = np.arange(128)[None, :]
    triadd = np.where(cc >= pp[:, None], 0.0, -1e30).astype(f)
    triadd = np.ascontiguousarray(triadd)

    ident = np.eye(128, dtype=f)

    def col128(b_):  # [512] -> [128, 4] (partition-major per 128-tile)
        return np.ascontiguousarray(np.asarray(b_, dtype=f).reshape(4, 128).T)

    in_maps = []
    for c in range(N_CORES):
        b, hg = c // 2, c % 2
        sl = slice(hg * 512, hg * 512 + 512)
        in_maps.append({
            "x": np.ascontiguousarray(x[b], dtype=f),
            "wq": np.ascontiguousarray(Wq[sl, :].T, dtype=f),
            "wk": np.ascontiguousarray(Wk[sl, :].T, dtype=f),
            "wv": np.ascontiguousarray(Wv[sl, :].T, dtype=f),
            "wo": np.ascontiguousarray(Wo[:, sl].T, dtype=f),
            "bq": col128(bq[sl]),
            "bk": col128(bk[sl]),
            "bv_bc": np.ascontiguousarray(
                np.tile(np.asarray(bv[sl], dtype=f)[None, :], (128, 1))),
            "cosT": cosT, "sinT": sinT, "ident": ident,
            "permT": permT, "triadd": triadd,
        })
    return in_maps


def _run(inputs, trace=False):
    from concourse import bass_utils

    if "nc" not in _cache:
        _cache["nc"] = _build_bass()
    nc = _cache["nc"]
    in_maps = _prep_shards(**inputs)
    # The remote device occasionally reports a transient unrecoverable
    # state right after loading a fresh NEFF; a retry reliably clears it.
    last_exc = None
    for _ in range(3):
        try:
            res = bass_utils.run_bass_kernel_spmd(
                nc, in_maps, core_ids=list(range(N_CORES)), trace=trace)
            break
        except Exception as e:  # noqa: BLE001
            last_exc = e
            import time
            time.sleep(2.0)
    else:
        raise last_exc

    bo = np.asarray(inputs["bo"], dtype=np.float32)
    out = np.empty((B, T, D), dtype=np.float32)
    for b in range(B):
        out[b] = (res.results[2 * b]["yT"].T
                  + res.results[2 * b + 1]["yT"].T + bo)
    return out, res


def kernel(**inputs):
    out, _ = _run(inputs, trace=False)
    return out



# revision 28
# speedup vs baseline: 1.0010x; 1.0010x over previous
"""Causal self-attention (RoPE) Trainium2 kernel.

Problem: B=4, T=2048, D=1024, H=16 heads (hd=64), fp32.
  q,k,v = x@W{q,k,v}.T + b;  rope(q), rope(k);  causal softmax attention;
  y = att_out @ Wo.T + bo.

Sharding (8 cores): data parallel over batch (4), tensor parallel over
heads (2 groups of 8 heads). Core c handles batch c//2, head-group c%2.
Each core computes its 8 heads end-to-end plus the partial out-projection;
the host sums the two head-group partials per batch and adds bo.

On-device layout is transposed ([dim, time]) so that attention matmuls get
the contraction dim (head dim / keys) on partitions:
  - x.T built via PE transposes
  - Q.T/K.T = W @ x.T directly; RoPE applied elementwise, with the
    rotate-half realized as a single +/-1 permutation-matrix matmul on PE
  - V projected per x-strip, interleaved with the transposes (PE filler
    while the next strip's DMA is in flight)
  - S.T = K_h @ Q_h.T per head-PAIR (even head at partitions 0-63, odd at
    64-127 — the two K=64 matmuls run concurrently in the PE array); one
    exp per pair over [128, 1024], diagonal tiles sliced to the valid
    region with a static [128,128] 0/-1e30 triangle added to the S-PSUM
    before exp (keeps at-tiles pure ACT->PE, no event-semaphore storms)
  - O.T = [V_h | 1].T @ A.T accumulated over key tiles; the ones column
    yields the softmax denominator as psum row 64; AV matmuls trail the
    S matmuls by LAG key-tile pairs, and each pair's trailing AVs,
    reciprocals and normalization tails are deferred into the next pair's
    instruction stream so the PE FIFO never drains at pair boundaries
  - normalization via reciprocal + K=1 broadcast matmul
  - out.T = Wo_c.T @ Y.T, emitted per finished query chunk (interleaved
    into the next chunk's attention), written transposed; host transposes

All big matmuls run in float32r (full-rate fp32, ~1e-4 relative rounding).
"""

import sys

sys.path.insert(0, "/opt/trn_rl_repo")

import numpy as np

B, T, D, H = 4, 2048, 1024, 16
HD = 64
ROPE_BASE = 10000.0
N_CORES = 8
HPC = 8  # heads per core
LAG = 5  # AV matmul lag behind S matmul (key-tile pairs)

_cache = {}


def _build_bass():
    import concourse.mybir as mybir
    import concourse.tile as tile
    from concourse import bacc

    f32 = mybir.dt.float32
    f32r = mybir.dt.float32r
    Alu = mybir.AluOpType
    Act = mybir.ActivationFunctionType

    nc = bacc.Bacc()

    # ---- DRAM I/O (per-core shards; same NEFF on all 8 cores) ----
    x_d = nc.dram_tensor("x", [T, D], f32r, kind="ExternalInput")
    wq_d = nc.dram_tensor("wq", [D, 512], f32r, kind="ExternalInput")
    wk_d = nc.dram_tensor("wk", [D, 512], f32r, kind="ExternalInput")
    wv_d = nc.dram_tensor("wv", [D, 512], f32r, kind="ExternalInput")
    wo_d = nc.dram_tensor("wo", [512, D], f32r, kind="ExternalInput")
    bq_d = nc.dram_tensor("bq", [128, 4], f32, kind="ExternalInput")
    bk_d = nc.dram_tensor("bk", [128, 4], f32, kind="ExternalInput")
    bv_d = nc.dram_tensor("bv_bc", [128, 512], f32, kind="ExternalInput")
    cos_d = nc.dram_tensor("cosT", [128, T], f32r, kind="ExternalInput")
    sin_d = nc.dram_tensor("sinT", [128, T], f32r, kind="ExternalInput")
    perm_d = nc.dram_tensor("permT", [128, 128], f32r, kind="ExternalInput")
    tri_d = nc.dram_tensor("triadd", [128, 128], f32, kind="ExternalInput")
    ident_d = nc.dram_tensor("ident", [128, 128], f32r, kind="ExternalInput")
    yt_d = nc.dram_tensor("yT", [D, T], f32, kind="ExternalOutput")

    wq_v = wq_d[:, :].rearrange("(ko p) m -> p ko m", p=128)
    wk_v = wk_d[:, :].rearrange("(ko p) m -> p ko m", p=128)
    wv_v = wv_d[:, :].rearrange("(ko p) m -> p ko m", p=128)
    wo_v = wo_d[:, :].rearrange("(ko p) n -> p ko n", p=128)

    with tile.TileContext(nc) as tc:
        with (
            tc.tile_pool(name="singles", bufs=1) as singles,
            tc.tile_pool(name="big", bufs=1) as big,
        ):
            ident = singles.tile([128, 128], f32r)
            nc.sync.dma_start(ident, ident_d[:, :])
            ones_hi = singles.tile([65, 64], f32)
            nc.vector.memset(ones_hi, 1.0)
            ones128 = singles.tile([128, 16, 8, 1], f32)
            nc.vector.memset(ones128, 1.0)
            # setup constants ride the otherwise-idle gpsimd DMA queue
            # (issued after the wv halves below; consumed only in A2/B)
            bq_sb = singles.tile([128, 4], f32, tag="bq")
            bk_sb = singles.tile([128, 4], f32, tag="bk")
            perm_sb = singles.tile([128, 128], f32r, tag="perm")
            tri_sb = singles.tile([128, 128], f32, tag="tri")

            # persistent activations (f32r so matmuls can consume them)
            qt = big.tile([128, 4, T], f32r, tag="qt")
            kt = big.tile([128, 4, T], f32r, tag="kt")
            v_sb = big.tile([128, 16, HPC, 65], f32r, tag="v")
            # ones columns for the fused softmax-denominator rows: written by
            # DVE (a 4-byte-element gather DMA would clog the DGE ring and
            # block the issuing engine's queue for ~20us)
            nc.vector.tensor_copy(v_sb[:, :, :, 64:65], ones128)

            # ================= Phase A: x.T, Q.T/K.T (roped), V =============
            with (
                tc.tile_pool(name="pa_sb", bufs=1) as pa,
                tc.tile_pool(name="xrow_p", bufs=4) as xrow_p,
                tc.tile_pool(name="wcol_p", bufs=2) as wcol_p,
                tc.tile_pool(name="qa_p", bufs=4) as qa_p,
                tc.tile_pool(name="tmp_p", bufs=2) as tmp_p,
                tc.tile_pool(name="tps", bufs=2, space="PSUM") as tps,
                tc.tile_pool(name="qkps", bufs=3, space="PSUM") as qkps,
                tc.tile_pool(name="auxps", bufs=3, space="PSUM") as auxps,
            ):
                # wv first: the V matmuls of strip 0 sit early in the PE
                # FIFO and must not wait behind the cos/sin table DMAs
                wv_sb = pa.tile([128, 8, 512], f32r, tag="wv")
                nc.scalar.dma_start(wv_sb[:, 0:4, :], wv_v[:, 0:4, :])
                nc.gpsimd.dma_start(wv_sb[:, 4:8, :], wv_v[:, 4:8, :])
                bv_sb = pa.tile([128, 512], f32, tag="bv")
                nc.gpsimd.dma_start(bv_sb, bv_d[:, :])
                nc.gpsimd.dma_start(bq_sb, bq_d[:, :])
                nc.gpsimd.dma_start(bk_sb, bk_d[:, :])
                nc.gpsimd.dma_start(perm_sb, perm_d[:, :])
                nc.gpsimd.dma_start(tri_sb, tri_d[:, :])
                # cos/sin are first consumed in A2 of th=0; issuing them
                # after the first strips keeps the DMA engines on x rows
                cos_sb = pa.tile([128, T], f32r, tag="cos")
                sin_sb = pa.tile([128, T], f32r, tag="sin")
                cs_issued = [False]

                for th in range(2):
                    t0 = th * 1024
                    xt = pa.tile([128, 8, 1024], f32r, tag="xt")
                    # --- A1: transpose x strips into xt; V projection of
                    # each strip follows immediately (PE filler while the
                    # next strip's DMA is in flight) ---
                    def do_v(tt, th=th, xt=xt):
                        gt = th * 8 + tt
                        psV = auxps.tile([128, 512], f32, tag="aux",
                                         name=f"psV_{gt}")
                        for ko in range(8):
                            nc.tensor.matmul(
                                psV, lhsT=xt[:, ko, tt * 128:(tt + 1) * 128],
                                rhs=wv_sb[:, ko, :],
                                start=(ko == 0), stop=(ko == 7))
                        nc.vector.tensor_tensor(
                            v_sb[:, gt, :, 0:64],
                            psV.rearrange("p (h d) -> p h d", h=HPC),
                            bv_sb.rearrange("p (h d) -> p h d", h=HPC),
                            Alu.add)

                    # V lags the transposes (3 strips on the cold start so
                    # early PE work never waits on the wv DMA)
                    vlag = 3 if th == 0 else 1
                    for tt in range(8):
                        r0 = t0 + tt * 128
                        xrow = xrow_p.tile([128, D], f32r, tag="xrow")
                        nc.sync.dma_start(xrow, x_d[r0:r0 + 128, :])
                        if th == 0 and tt == 3 and not cs_issued[0]:
                            cs_issued[0] = True
                            nc.scalar.dma_start(cos_sb, cos_d[:, :])
                            nc.scalar.dma_start(sin_sb, sin_d[:, :])
                        for ko in range(8):
                            tp = tps.tile([128, 128], f32r, tag="tps")
                            nc.tensor.transpose(
                                tp, xrow[:, ko * 128:(ko + 1) * 128], ident)
                            nc.scalar.copy(
                                xt[:, ko, tt * 128:(tt + 1) * 128], tp)
                        if tt >= vlag:
                            do_v(tt - vlag)
                    for tt in range(8 - vlag, 8):
                        do_v(tt)

                    # --- A2: Q.T and K.T with fused RoPE ---
                    for wview, bcol, dest in (
                        (wq_v, bq_sb, qt),
                        (wk_v, bk_sb, kt),
                    ):
                        for qo in range(4):
                            wcol = wcol_p.tile([128, 8, 128], f32r, tag="wcol")
                            nc.sync.dma_start(
                                wcol, wview[:, :, qo * 128:(qo + 1) * 128])
                            # two 512-chunks in flight so the rot matmul's
                            # DVE dependency hides under the other chunk's
                            # accumulation matmuls
                            ps_l, qa_l = [], []
                            for tcc in range(2):
                                c0 = tcc * 512
                                psA = qkps.tile([128, 512], f32, tag="psA")
                                for ko in range(8):
                                    nc.tensor.matmul(
                                        psA, lhsT=wcol[:, ko, :],
                                        rhs=xt[:, ko, c0:c0 + 512],
                                        start=(ko == 0), stop=(ko == 7))
                                qa = qa_p.tile([128, 512], f32r, tag="qa")
                                nc.vector.tensor_scalar_add(
                                    qa, psA, bcol[:, qo:qo + 1])
                                ps_l.append(psA)
                                qa_l.append(qa)
                            rot_l = []
                            for tcc in range(2):
                                rps = auxps.tile([128, 512], f32, tag="aux")
                                nc.tensor.matmul(
                                    rps, lhsT=perm_sb, rhs=qa_l[tcc],
                                    start=True, stop=True)
                                rot_l.append(rps)
                            for tcc in range(2):
                                ta = t0 + tcc * 512
                                # tmp1 is SBUF-only -> pool engine (gpsimd);
                                # tmp2 reads PSUM so it must stay on DVE
                                tmp1 = tmp_p.tile([128, 512], f32, tag="tmp1")
                                nc.gpsimd.tensor_tensor(
                                    tmp1, qa_l[tcc], cos_sb[:, ta:ta + 512],
                                    Alu.mult)
                                tmp2 = tmp_p.tile([128, 512], f32, tag="tmp2")
                                nc.vector.tensor_tensor(
                                    tmp2, rot_l[tcc], sin_sb[:, ta:ta + 512],
                                    Alu.mult)
                                nc.vector.tensor_tensor(
                                    dest[:, qo, ta:ta + 512], tmp1, tmp2,
                                    Alu.add)


            # ================= Phase B: attention ==========================
            with tc.tile_pool(name="pb_keep", bufs=1) as pb_keep:
                yt = pb_keep.tile([128, 4, T], f32r, tag="yt")
                wo_sb = pb_keep.tile([128, 4, D], f32r, tag="wo")
                nc.scalar.dma_start(wo_sb, wo_v)
                with (
                    tc.tile_pool(name="at_p", bufs=7) as at_p,
                    tc.tile_pool(name="rec_p", bufs=4) as rec_p,
                    tc.tile_pool(name="ytmp_p", bufs=2) as ytmp_p,
                    tc.tile_pool(name="orow_p", bufs=4) as orow_p,
                    tc.tile_pool(name="sps", bufs=2, space="PSUM") as sps,
                    tc.tile_pool(name="ops", bufs=4, space="PSUM") as ops,
                ):
                    def emit_outproj(cj, dos=range(8)):
                        p0 = cj * 512
                        for do in dos:
                            ps2 = sps.tile([128, 1024], f32, tag="sps",
                                           name=f"op_{cj}_{do}")
                            ps = ps2[:, 0:512]
                            for ko in range(4):
                                nc.tensor.matmul(
                                    ps, lhsT=wo_sb[:, ko,
                                                   do * 128:(do + 1) * 128],
                                    rhs=yt[:, ko, p0:p0 + 512],
                                    start=(ko == 0), stop=(ko == 3))
                            orow = orow_p.tile([128, 512], f32, tag="orow")
                            nc.vector.tensor_copy(orow, ps)
                            nc.sync.dma_start(
                                yt_d[do * 128:(do + 1) * 128, p0:p0 + 512],
                                orow)

                    pending = []
                    pending_avs = []
                    pending_recips = []

                    def flush_avs():
                        for fn in pending_avs:
                            fn()
                        pending_avs.clear()
                        for fn in pending_recips:
                            fn()
                        pending_recips.clear()

                    def flush_tails():
                        flush_avs()
                        for fn in pending:
                            fn()
                        pending.clear()

                    for ci in range(4):
                        q0 = ci * 512
                        nkt = 4 * ci + 4
                        for ho in range(4):
                            if ho == 1 and ci > 0:
                                flush_tails()
                                emit_outproj(ci - 1, range(0, 4))
                            elif ho == 2 and ci > 0:
                                emit_outproj(ci - 1, range(4, 8))
                            o_pair = [
                                ops.tile([65, 512], f32, tag="ops",
                                         name=f"ops_{ci}_{ho}_{g_}")
                                for g_ in range(2)]
                            at2s = {}

                            def do_av(k_i, o_pair=o_pair, at2s=at2s, nkt=nkt,
                                      ho=ho, ci=ci):
                                at2 = at2s.pop(k_i)
                                sdx = k_i - 4 * ci
                                f0 = max(0, sdx) * 128
                                for g in range(2):
                                    nc.tensor.matmul(
                                        o_pair[g][:, f0:512],
                                        lhsT=v_sb[:, k_i, 2 * ho + g, :],
                                        rhs=at2[:, g * 512 + f0:
                                                (g + 1) * 512],
                                        start=(k_i == 0),
                                        stop=(k_i == nkt - 1))

                            for kt_i in range(nkt):
                                sdx = kt_i - 4 * ci
                                f0 = max(0, sdx) * 128
                                s_ps2 = sps.tile([128, 1024], f32, tag="sps")
                                for g in range(2):
                                    hp = g * 64
                                    nc.tensor.matmul(
                                        s_ps2[:, g * 512 + f0:(g + 1) * 512],
                                        lhsT=kt[hp:hp + 64, ho,
                                                kt_i * 128:(kt_i + 1) * 128],
                                        rhs=qt[hp:hp + 64, ho,
                                               q0 + f0:q0 + 512],
                                        start=True, stop=True)
                                at2 = at_p.tile([128, 1024], f32r, tag="at")
                                av = at2.rearrange("p (g q) -> p g q", g=2)
                                sv = s_ps2.rearrange("p (g q) -> p g q", g=2)
                                if sdx >= 0:
                                    nc.vector.tensor_tensor(
                                        sv[:, :, f0:f0 + 128],
                                        sv[:, :, f0:f0 + 128],
                                        tri_sb[:, None, :].to_broadcast(
                                            (128, 2, 128)),
                                        Alu.add)
                                nc.scalar.activation(
                                    av[:, :, f0:512], sv[:, :, f0:512],
                                    Act.Exp, scale=0.125)
                                at2s[kt_i] = at2
                                if kt_i == 0:
                                    flush_avs()
                                elif kt_i == 1:
                                    flush_tails()
                                if kt_i >= LAG:
                                    do_av(kt_i - LAG)
                            for k_i in range(max(0, nkt - LAG), nkt):
                                pending_avs.append(
                                    lambda k_i=k_i, do_av=do_av: do_av(k_i))

                            recs = [rec_p.tile([65, 512], f32, tag="rec",
                                               name=f"rec_{ci}_{ho}_{g_}")
                                    for g_ in range(2)]
                            for g in range(2):
                                o_ps = o_pair[g]
                                rec = recs[g]

                                def recip(o_ps=o_ps, rec=rec):
                                    # approx reciprocal computes garbage on
                                    # APs not based at partition 0, so run it
                                    # over all 65 rows (same cost: DVE time
                                    # scales with the free dim only). Rows
                                    # 0-63 are junk and never read; row 64 is
                                    # the softmax denominator.
                                    nc.vector.reciprocal_approx_fast(
                                        out=rec, in_=o_ps)

                                pending_recips.append(recip)

                                def rest(g=g, o_ps=o_ps, rec=rec, ho=ho,
                                         q0=q0, ci=ci):
                                    b_ps = ops.tile(
                                        [64, 512], f32, tag="ops",
                                        name=f"bps_{ci}_{ho}_{g}")
                                    nc.tensor.matmul(
                                        b_ps, lhsT=ones_hi[64:65, :],
                                        rhs=rec[64:65, :],
                                        start=True, stop=True)
                                    b_sb2 = rec_p.tile(
                                        [64, 512], f32, tag="bsb")
                                    nc.vector.tensor_copy(b_sb2, b_ps)
                                    if g == 0:
                                        nc.vector.tensor_tensor(
                                            yt[0:64, ho, q0:q0 + 512],
                                            o_ps[0:64, :], b_sb2, Alu.mult)
                                    else:
                                        ytmp = ytmp_p.tile(
                                            [64, 512], f32r, tag="ytmp")
                                        nc.vector.tensor_tensor(
                                            ytmp, o_ps[0:64, :], b_sb2,
                                            Alu.mult)
                                        nc.sync.dma_start(
                                            yt[64:128, ho, q0:q0 + 512], ytmp)

                                pending.append(rest)

                    flush_tails()
                    emit_outproj(3)

    nc.finalize()
    return nc


def _prep_shards(x, Wq, bq, Wk, bk, Wv, bv, Wo, bo):
    f = np.float32
    theta = 1.0 / (ROPE_BASE ** (np.arange(0, HD, 2, dtype=f) / HD))  # [32]
    pos = np.arange(1, T + 1, dtype=f)
    ang = pos[:, None] * theta[None, :]  # [T, 32]
    j = (np.arange(128) % HD) % 32
    cosT = np.ascontiguousarray(np.cos(ang).T[j, :], dtype=f)  # [128, T]
    sinT = np.ascontiguousarray(np.sin(ang).T[j, :], dtype=f)
    # rotate-half permutation (with sign): rot[p] = sgn(p) * q[p ^ 32]
    prm = np.zeros((128, 128), dtype=f)
    pp = np.arange(128)
    prm[pp, pp ^ 32] = np.where((pp % HD) < 32, -1.0, 1.0)
    permT = np.ascontiguousarray(prm.T)

    # additive causal mask for the diagonal 128-block: keep c >= p
    cc = np.arange(128)[None, :]
    triadd = np.where(cc >= pp[:, None], 0.0, -1e30).astype(f)
    triadd = np.ascontiguousarray(triadd)

    ident = np.eye(128, dtype=f)

    def col128(b_):  # [512] -> [128, 4] (partition-major per 128-tile)
        return np.ascontiguousarray(np.asarray(b_, dtype=f).reshape(4, 128).T)

    in_maps = []
    for c in range(N_CORES):
        b, hg = c // 2, c % 2
        sl = slice(hg * 512, hg * 512 + 512)
        in_maps.append({
            "x": np.ascontiguousarray(x[b], dtype=f),
            "wq": np.ascontiguousarray(Wq[sl, :].T, dtype=f),
            "wk": np.ascontiguousarray(Wk[sl, :].T, dtype=f),
            "wv": np.ascontiguousarray(Wv[sl, :].T, dtype=f),
            "wo": np.ascontiguousarray(Wo[:, sl].T, dtype=f),
            "bq": col128(bq[sl]),
            "bk": col128(bk[sl]),
            "bv_bc": np.ascontiguousarray(
                np.tile(np.asarray(bv[sl], dtype=f)[None, :], (128, 1))),
            "cosT": cosT, "sinT": sinT, "ident": ident,
            "permT": permT, "triadd": triadd,
        })
    return in_maps


def _run(inputs, trace=False):
    from concourse import bass_utils

    if "nc" not in _cache:
        _cache["nc"] = _build_bass()
    nc = _cache["nc"]
    in_maps = _prep_shards(**inputs)
    # The remote device occasionally reports a transient unrecoverable
    # state right after loading a fresh NEFF; a retry reliably clears it.
    last_exc = None
    for _ in range(3):
        try:
            res = bass_utils.run_bass_kernel_spmd(
                nc, in_maps, core_ids=list(range(N_CORES)), trace=trace)
            break
        except Exception as e:  # noqa: BLE001
            last_exc = e
            import time
            time.sleep(2.0)
    else:
        raise last_exc

    bo = np.asarray(inputs["bo"], dtype=np.float32)
    out = np.empty((B, T, D), dtype=np.float32)
    for b in range(B):
        out[b] = (res.results[2 * b]["yT"].T
                  + res.results[2 * b + 1]["yT"].T + bo)
    return out, res


def kernel(**inputs):
    out, _ = _run(inputs, trace=False)
    return out



# revision 29
# speedup vs baseline: 1.0306x; 1.0296x over previous
"""Causal self-attention (RoPE) Trainium2 kernel.

Problem: B=4, T=2048, D=1024, H=16 heads (hd=64), fp32.
  q,k,v = x@W{q,k,v}.T + b;  rope(q), rope(k);  causal softmax attention;
  y = att_out @ Wo.T + bo.

Sharding (8 cores): data parallel over batch (4), tensor parallel over
heads (2 groups of 8 heads). Core c handles batch c//2, head-group c%2.
Each core computes its 8 heads end-to-end plus the partial out-projection;
the host sums the two head-group partials per batch and adds bo.

On-device layout is transposed ([dim, time]) so that attention matmuls get
the contraction dim (head dim / keys) on partitions:
  - x.T built via PE transposes
  - Q.T/K.T = W @ x.T directly; RoPE applied elementwise, with the
    rotate-half realized as a single +/-1 permutation-matrix matmul on PE
  - V projected per x-strip, interleaved with the transposes (PE filler
    while the next strip's DMA is in flight)
  - S.T = K_h @ Q_h.T per head-PAIR (even head at partitions 0-63, odd at
    64-127 — the two K=64 matmuls run concurrently in the PE array); one
    exp per pair over [128, 1024], diagonal tiles sliced to the valid
    region with a static [128,128] 0/-1e30 triangle added to the S-PSUM
    before exp (keeps at-tiles pure ACT->PE, no event-semaphore storms)
  - O.T = [V_h | 1].T @ A.T accumulated over key tiles; the ones column
    yields the softmax denominator as psum row 64; AV matmuls trail the
    S matmuls by LAG key-tile pairs, and each pair's trailing AVs,
    reciprocals and normalization tails are deferred into the next pair's
    instruction stream so the PE FIFO never drains at pair boundaries
  - normalization via reciprocal + K=1 broadcast matmul
  - out.T = Wo_c.T @ Y.T, emitted per finished query chunk (interleaved
    into the next chunk's attention), written transposed; host transposes

All big matmuls run in float32r (full-rate fp32, ~1e-4 relative rounding).
"""

import sys

sys.path.insert(0, "/opt/trn_rl_repo")

import numpy as np

B, T, D, H = 4, 2048, 1024, 16
HD = 64
ROPE_BASE = 10000.0
N_CORES = 8
HPC = 8  # heads per core
LAG = 5  # AV matmul lag behind S matmul (key-tile pairs)

_cache = {}


def _build_bass():
    import concourse.mybir as mybir
    import concourse.tile as tile
    from concourse import bacc

    f32 = mybir.dt.float32
    f32r = mybir.dt.float32r
    Alu = mybir.AluOpType
    Act = mybir.ActivationFunctionType

    nc = bacc.Bacc()

    # ---- DRAM I/O (per-core shards; same NEFF on all 8 cores) ----
    x_d = nc.dram_tensor("x", [T, D], f32r, kind="ExternalInput")
    wq_d = nc.dram_tensor("wq", [D, 512], f32r, kind="ExternalInput")
    wk_d = nc.dram_tensor("wk", [D, 512], f32r, kind="ExternalInput")
    wv_d = nc.dram_tensor("wv", [D, 512], f32r, kind="ExternalInput")
    wo_d = nc.dram_tensor("wo", [512, D], f32r, kind="ExternalInput")
    bq_d = nc.dram_tensor("bq", [128, 4], f32, kind="ExternalInput")
    bk_d = nc.dram_tensor("bk", [128, 4], f32, kind="ExternalInput")
    bv_d = nc.dram_tensor("bv_bc", [128, 512], f32, kind="ExternalInput")
    cos_d = nc.dram_tensor("cosT", [128, T], f32r, kind="ExternalInput")
    sin_d = nc.dram_tensor("sinT", [128, T], f32r, kind="ExternalInput")
    perm_d = nc.dram_tensor("permT", [128, 128], f32r, kind="ExternalInput")
    tri_d = nc.dram_tensor("triadd", [128, 128], f32, kind="ExternalInput")
    ident_d = nc.dram_tensor("ident", [128, 128], f32r, kind="ExternalInput")
    yt_d = nc.dram_tensor("yT", [D, T], f32, kind="ExternalOutput")

    wq_v = wq_d[:, :].rearrange("(ko p) m -> p ko m", p=128)
    wk_v = wk_d[:, :].rearrange("(ko p) m -> p ko m", p=128)
    wv_v = wv_d[:, :].rearrange("(ko p) m -> p ko m", p=128)
    wo_v = wo_d[:, :].rearrange("(ko p) n -> p ko n", p=128)

    with tile.TileContext(nc) as tc:
        with (
            tc.tile_pool(name="singles", bufs=1) as singles,
            tc.tile_pool(name="big", bufs=1) as big,
        ):
            ident = singles.tile([128, 128], f32r)
            nc.sync.dma_start(ident, ident_d[:, :])
            ones_hi = singles.tile([65, 64], f32)
            nc.vector.memset(ones_hi, 1.0)
            ones128 = singles.tile([128, 16, 8, 1], f32)
            nc.vector.memset(ones128, 1.0)
            # setup constants ride the otherwise-idle gpsimd DMA queue so
            # the sync queue's x strips get the DMA engines first
            bq_sb = singles.tile([128, 4], f32, tag="bq")
            nc.gpsimd.dma_start(bq_sb, bq_d[:, :])
            bk_sb = singles.tile([128, 4], f32, tag="bk")
            nc.gpsimd.dma_start(bk_sb, bk_d[:, :])
            perm_sb = singles.tile([128, 128], f32r, tag="perm")
            nc.gpsimd.dma_start(perm_sb, perm_d[:, :])
            tri_sb = singles.tile([128, 128], f32, tag="tri")
            nc.gpsimd.dma_start(tri_sb, tri_d[:, :])

            # persistent activations (f32r so matmuls can consume them)
            qt = big.tile([128, 4, T], f32r, tag="qt")
            kt = big.tile([128, 4, T], f32r, tag="kt")
            v_sb = big.tile([128, 16, HPC, 65], f32r, tag="v")
            # ones columns for the fused softmax-denominator rows: written by
            # DVE (a 4-byte-element gather DMA would clog the DGE ring and
            # block the issuing engine's queue for ~20us)
            nc.vector.tensor_copy(v_sb[:, :, :, 64:65], ones128)

            # ================= Phase A: x.T, Q.T/K.T (roped), V =============
            with (
                tc.tile_pool(name="pa_sb", bufs=1) as pa,
                tc.tile_pool(name="xrow_p", bufs=4) as xrow_p,
                tc.tile_pool(name="wcol_p", bufs=2) as wcol_p,
                tc.tile_pool(name="qa_p", bufs=4) as qa_p,
                tc.tile_pool(name="tmp_p", bufs=2) as tmp_p,
                tc.tile_pool(name="tps", bufs=2, space="PSUM") as tps,
                tc.tile_pool(name="qkps", bufs=3, space="PSUM") as qkps,
                tc.tile_pool(name="auxps", bufs=3, space="PSUM") as auxps,
            ):
                # wv first: the V matmuls of strip 0 sit early in the PE
                # FIFO and must not wait behind the cos/sin table DMAs
                wv_sb = pa.tile([128, 8, 512], f32r, tag="wv")
                nc.scalar.dma_start(wv_sb[:, 0:4, :], wv_v[:, 0:4, :])
                nc.gpsimd.dma_start(wv_sb[:, 4:8, :], wv_v[:, 4:8, :])
                bv_sb = pa.tile([128, 512], f32, tag="bv")
                nc.gpsimd.dma_start(bv_sb, bv_d[:, :])
                # cos/sin are first consumed in A2 of th=0; issuing them
                # after the first strips keeps the DMA engines on x rows
                cos_sb = pa.tile([128, T], f32r, tag="cos")
                sin_sb = pa.tile([128, T], f32r, tag="sin")
                cs_issued = [False]

                for th in range(2):
                    t0 = th * 1024
                    xt = pa.tile([128, 8, 1024], f32r, tag="xt")
                    # --- A1: transpose x strips into xt; V projection of
                    # each strip follows immediately (PE filler while the
                    # next strip's DMA is in flight) ---
                    def do_v(tt, th=th, xt=xt):
                        gt = th * 8 + tt
                        psV = auxps.tile([128, 512], f32, tag="aux",
                                         name=f"psV_{gt}")
                        for ko in range(8):
                            nc.tensor.matmul(
                                psV, lhsT=xt[:, ko, tt * 128:(tt + 1) * 128],
                                rhs=wv_sb[:, ko, :],
                                start=(ko == 0), stop=(ko == 7))
                        nc.vector.tensor_tensor(
                            v_sb[:, gt, :, 0:64],
                            psV.rearrange("p (h d) -> p h d", h=HPC),
                            bv_sb.rearrange("p (h d) -> p h d", h=HPC),
                            Alu.add)

                    # V lags the transposes (3 strips on the cold start so
                    # early PE work never waits on the wv DMA)
                    vlag = 3 if th == 0 else 1
                    for tt in range(8):
                        r0 = t0 + tt * 128
                        xrow = xrow_p.tile([128, D], f32r, tag="xrow")
                        nc.sync.dma_start(xrow, x_d[r0:r0 + 128, :])
                        if th == 0 and tt == 3 and not cs_issued[0]:
                            cs_issued[0] = True
                            nc.scalar.dma_start(cos_sb, cos_d[:, :])
                            nc.scalar.dma_start(sin_sb, sin_d[:, :])
                        for ko in range(8):
                            tp = tps.tile([128, 128], f32r, tag="tps")
                            nc.tensor.transpose(
                                tp, xrow[:, ko * 128:(ko + 1) * 128], ident)
                            nc.scalar.copy(
                                xt[:, ko, tt * 128:(tt + 1) * 128], tp)
                        if tt >= vlag:
                            do_v(tt - vlag)
                    for tt in range(8 - vlag, 8):
                        do_v(tt)

                    # --- A2: Q.T and K.T with fused RoPE ---
                    for wview, bcol, dest in (
                        (wq_v, bq_sb, qt),
                        (wk_v, bk_sb, kt),
                    ):
                        for qo in range(4):
                            wcol = wcol_p.tile([128, 8, 128], f32r, tag="wcol")
                            nc.sync.dma_start(
                                wcol, wview[:, :, qo * 128:(qo + 1) * 128])
                            # two 512-chunks in flight so the rot matmul's
                            # DVE dependency hides under the other chunk's
                            # accumulation matmuls
                            ps_l, qa_l = [], []
                            for tcc in range(2):
                                c0 = tcc * 512
                                psA = qkps.tile([128, 512], f32, tag="psA")
                                for ko in range(8):
                                    nc.tensor.matmul(
                                        psA, lhsT=wcol[:, ko, :],
                                        rhs=xt[:, ko, c0:c0 + 512],
                                        start=(ko == 0), stop=(ko == 7))
                                qa = qa_p.tile([128, 512], f32r, tag="qa")
                                nc.vector.tensor_scalar_add(
                                    qa, psA, bcol[:, qo:qo + 1])
                                ps_l.append(psA)
                                qa_l.append(qa)
                            rot_l = []
                            for tcc in range(2):
                                rps = auxps.tile([128, 512], f32, tag="aux")
                                nc.tensor.matmul(
                                    rps, lhsT=perm_sb, rhs=qa_l[tcc],
                                    start=True, stop=True)
                                rot_l.append(rps)
                            for tcc in range(2):
                                ta = t0 + tcc * 512
                                # tmp1 is SBUF-only -> pool engine (gpsimd);
                                # tmp2 reads PSUM so it must stay on DVE
                                tmp1 = tmp_p.tile([128, 512], f32, tag="tmp1")
                                nc.gpsimd.tensor_tensor(
                                    tmp1, qa_l[tcc], cos_sb[:, ta:ta + 512],
                                    Alu.mult)
                                tmp2 = tmp_p.tile([128, 512], f32, tag="tmp2")
                                nc.vector.tensor_tensor(
                                    tmp2, rot_l[tcc], sin_sb[:, ta:ta + 512],
                                    Alu.mult)
                                nc.vector.tensor_tensor(
                                    dest[:, qo, ta:ta + 512], tmp1, tmp2,
                                    Alu.add)


            # ================= Phase B: attention ==========================
            with tc.tile_pool(name="pb_keep", bufs=1) as pb_keep:
                yt = pb_keep.tile([128, 4, T], f32r, tag="yt")
                wo_sb = pb_keep.tile([128, 4, D], f32r, tag="wo")
                nc.scalar.dma_start(wo_sb, wo_v)
                with (
                    tc.tile_pool(name="at_p", bufs=7) as at_p,
                    tc.tile_pool(name="rec_p", bufs=4) as rec_p,
                    tc.tile_pool(name="ytmp_p", bufs=2) as ytmp_p,
                    tc.tile_pool(name="orow_p", bufs=4) as orow_p,
                    tc.tile_pool(name="sps", bufs=2, space="PSUM") as sps,
                    tc.tile_pool(name="ops", bufs=4, space="PSUM") as ops,
                ):
                    def emit_outproj(cj, dos=range(8)):
                        p0 = cj * 512
                        for do in dos:
                            ps2 = sps.tile([128, 1024], f32, tag="sps",
                                           name=f"op_{cj}_{do}")
                            ps = ps2[:, 0:512]
                            for ko in range(4):
                                nc.tensor.matmul(
                                    ps, lhsT=wo_sb[:, ko,
                                                   do * 128:(do + 1) * 128],
                                    rhs=yt[:, ko, p0:p0 + 512],
                                    start=(ko == 0), stop=(ko == 3))
                            orow = orow_p.tile([128, 512], f32, tag="orow")
                            nc.vector.tensor_copy(orow, ps)
                            nc.sync.dma_start(
                                yt_d[do * 128:(do + 1) * 128, p0:p0 + 512],
                                orow)

                    pending = []
                    pending_avs = []
                    pending_recips = []

                    def flush_avs():
                        for fn in pending_avs:
                            fn()
                        pending_avs.clear()
                        for fn in pending_recips:
                            fn()
                        pending_recips.clear()

                    def flush_tails():
                        flush_avs()
                        for fn in pending:
                            fn()
                        pending.clear()

                    for ci in range(4):
                        q0 = ci * 512
                        nkt = 4 * ci + 4
                        for ho in range(4):
                            if ho == 1 and ci > 0:
                                flush_tails()
                                emit_outproj(ci - 1, range(0, 4))
                            elif ho == 2 and ci > 0:
                                emit_outproj(ci - 1, range(4, 8))
                            o_pair = [
                                ops.tile([65, 512], f32, tag="ops",
                                         name=f"ops_{ci}_{ho}_{g_}")
                                for g_ in range(2)]
                            at2s = {}

                            def do_av(k_i, o_pair=o_pair, at2s=at2s, nkt=nkt,
                                      ho=ho, ci=ci):
                                at2 = at2s.pop(k_i)
                                sdx = k_i - 4 * ci
                                f0 = max(0, sdx) * 128
                                for g in range(2):
                                    nc.tensor.matmul(
                                        o_pair[g][:, f0:512],
                                        lhsT=v_sb[:, k_i, 2 * ho + g, :],
                                        rhs=at2[:, g * 512 + f0:
                                                (g + 1) * 512],
                                        start=(k_i == 0),
                                        stop=(k_i == nkt - 1))

                            for kt_i in range(nkt):
                                sdx = kt_i - 4 * ci
                                f0 = max(0, sdx) * 128
                                s_ps2 = sps.tile([128, 1024], f32, tag="sps")
                                for g in range(2):
                                    hp = g * 64
                                    nc.tensor.matmul(
                                        s_ps2[:, g * 512 + f0:(g + 1) * 512],
                                        lhsT=kt[hp:hp + 64, ho,
                                                kt_i * 128:(kt_i + 1) * 128],
                                        rhs=qt[hp:hp + 64, ho,
                                               q0 + f0:q0 + 512],
                                        start=True, stop=True)
                                at2 = at_p.tile([128, 1024], f32r, tag="at")
                                av = at2.rearrange("p (g q) -> p g q", g=2)
                                sv = s_ps2.rearrange("p (g q) -> p g q", g=2)
                                if sdx >= 0:
                                    nc.vector.tensor_tensor(
                                        sv[:, :, f0:f0 + 128],
                                        sv[:, :, f0:f0 + 128],
                                        tri_sb[:, None, :].to_broadcast(
                                            (128, 2, 128)),
                                        Alu.add)
                                nc.scalar.activation(
                                    av[:, :, f0:512], sv[:, :, f0:512],
                                    Act.Exp, scale=0.125)
                                at2s[kt_i] = at2
                                if kt_i == 0:
                                    flush_avs()
                                elif kt_i == 1:
                                    flush_tails()
                                if kt_i >= LAG:
                                    do_av(kt_i - LAG)
                            for k_i in range(max(0, nkt - LAG), nkt):
                                pending_avs.append(
                                    lambda k_i=k_i, do_av=do_av: do_av(k_i))

                            recs = [rec_p.tile([65, 512], f32, tag="rec",
                                               name=f"rec_{ci}_{ho}_{g_}")
                                    for g_ in range(2)]
                            for g in range(2):
                                o_ps = o_pair[g]
                                rec = recs[g]

                                def recip(o_ps=o_ps, rec=rec):
                                    # approx reciprocal computes garbage on
                                    # APs not based at partition 0, so run it
                                    # over all 65 rows (same cost: DVE time
                                    # scales with the free dim only). Rows
                                    # 0-63 are junk and never read; row 64 is
                                    # the softmax denominator.
                                    nc.vector.reciprocal_approx_fast(
                                        out=rec, in_=o_ps)

                                pending_recips.append(recip)

                                def rest(g=g, o_ps=o_ps, rec=rec, ho=ho,
                                         q0=q0, ci=ci):
                                    b_ps = ops.tile(
                                        [64, 512], f32, tag="ops",
                                        name=f"bps_{ci}_{ho}_{g}")
                                    nc.tensor.matmul(
                                        b_ps, lhsT=ones_hi[64:65, :],
                                        rhs=rec[64:65, :],
                                        start=True, stop=True)
                                    b_sb2 = rec_p.tile(
                                        [64, 512], f32, tag="bsb")
                                    nc.vector.tensor_copy(b_sb2, b_ps)
                                    if g == 0:
                                        nc.vector.tensor_tensor(
                                            yt[0:64, ho, q0:q0 + 512],
                                            o_ps[0:64, :], b_sb2, Alu.mult)
                                    else:
                                        ytmp = ytmp_p.tile(
                                            [64, 512], f32r, tag="ytmp")
                                        nc.vector.tensor_tensor(
                                            ytmp, o_ps[0:64, :], b_sb2,
                                            Alu.mult)
                                        nc.sync.dma_start(
                                            yt[64:128, ho, q0:q0 + 512], ytmp)

                                pending.append(rest)

                    flush_tails()
                    emit_outproj(3)

    nc.finalize()
    return nc


def _prep_shards(x, Wq, bq, Wk, bk, Wv, bv, Wo, bo):
    f = np.float32
    theta = 1.0 / (ROPE_BASE ** (np.arange(0, HD, 2, dtype=f) / HD))  # [32]
    pos = np.arange(1, T + 1, dtype=f)
    ang = pos[:, None] * theta[None, :]  # [T, 32]
    j = (np.arange(128) % HD) % 32
    cosT = np.ascontiguousarray(np.cos(ang).T[j, :], dtype=f)  # [128, T]
    sinT = np.ascontiguousarray(np.sin(ang).T[j, :], dtype=f)
    # rotate-half permutation (with sign): rot[p] = sgn(p) * q[p ^ 32]
    prm = np.zeros((128, 128), dtype=f)
    pp = np.arange(128)
    prm[pp, pp ^ 32] = np.where((pp % HD) < 32, -1.0, 1.0)
    permT = np.ascontiguousarray(prm.T)

    # additive causal mask for the diagonal 128-block: keep c >= p
    cc = np.arange(128)[None, :]
    triadd = np.where(cc >= pp[:, None], 0.0, -1e30).astype(f)
    triadd = np.ascontiguousarray(triadd)

    ident = np.eye(128, dtype=f)

    def col128(b_):  # [512] -> [128, 4] (partition-major per 128-tile)
        return np.ascontiguousarray(np.asarray(b_, dtype=f).reshape(4, 128).T)

    in_maps = []
    for c in range(N_CORES):
        b, hg = c // 2, c % 2
        sl = slice(hg * 512, hg * 512 + 512)
        in_maps.append({
            "x": np.ascontiguousarray(x[b], dtype=f),
            "wq": np.ascontiguousarray(Wq[sl, :].T, dtype=f),
            "wk": np.ascontiguousarray(Wk[sl, :].T, dtype=f),
            "wv": np.ascontiguousarray(Wv[sl, :].T, dtype=f),
            "wo": np.ascontiguousarray(Wo[:, sl].T, dtype=f),
            "bq": col128(bq[sl]),
            "bk": col128(bk[sl]),
            "bv_bc": np.ascontiguousarray(
                np.tile(np.asarray(bv[sl], dtype=f)[None, :], (128, 1))),
            "cosT": cosT, "sinT": sinT, "ident": ident,
            "permT": permT, "triadd": triadd,
        })
    return in_maps


def _run(inputs, trace=False):
    from concourse import bass_utils

    if "nc" not in _cache:
        _cache["nc"] = _build_bass()
    nc = _cache["nc"]
    in_maps = _prep_shards(**inputs)
    # The remote device occasionally reports a transient unrecoverable
    # state right after loading a fresh NEFF; a retry reliably clears it.
    last_exc = None
    for _ in range(3):
        try:
            res = bass_utils.run_bass_kernel_spmd(
                nc, in_maps, core_ids=list(range(N_CORES)), trace=trace)
            break
        except Exception as e:  # noqa: BLE001
            last_exc = e
            import time
            time.sleep(2.0)
    else:
        raise last_exc

    bo = np.asarray(inputs["bo"], dtype=np.float32)
    out = np.empty((B, T, D), dtype=np.float32)
    for b in range(B):
        out[b] = (res.results[2 * b]["yT"].T
                  + res.results[2 * b + 1]["yT"].T + bo)
    return out, res


def kernel(**inputs):
    out, _ = _run(inputs, trace=False)
    return out

